# revision 27
# baseline (speedup 1.0000x reference)
"""CompoundHeadAttention TRN2 kernel (v2 — software-pipelined schedule).

Full-input contract: kernel(**inputs) takes the unsharded tensors from
setup_inputs() and returns the full [1, 2048, 2048] float32 output.

Sharding (8 cores, tensor-parallel over the HK=8 kv heads):
  core h owns kv head h: its Wq/Wk/Wv column slice, its WG[h]/bG[h], and
  Wfc row-slice [h*256:(h+1)*256, :].  Each core computes its head's
  attention + its partial FC output [2048, 2048]; the host sums the 8
  partials and adds bfc (the "all-reduce" of the row-sharded FC).

v2 schedule (vs v1): the PE stream is kept dense — scores (ST) run two
chunks ahead of the PV consumer so the ACT exp latency never stalls the
in-order PE queue; projection/G/FC matmuls are interleaved into the
attention stream as filler at a per-window rate; softmax denominators
use reciprocal_approx_fast (0.7us vs 4us); input DMAs are batched
(2 e-chunks per instr) and issued from both the Sync and ACT queues.
"""

import os
import sys
from collections import deque

import numpy as np

if "/opt/trn_rl_repo" not in sys.path and os.path.isdir("/opt/trn_rl_repo"):
    sys.path.insert(0, "/opt/trn_rl_repo")

import concourse.bass as bass  # noqa: E402
import concourse.mybir as mybir  # noqa: E402
import concourse.tile as tile  # noqa: E402
from concourse import bacc  # noqa: E402
from concourse import bass_utils  # noqa: E402

F32 = mybir.dt.float32
F32R = mybir.dt.float32r
F16 = mybir.dt.float16
AF = mybir.ActivationFunctionType

N = 2048
E = 2048
HK = 8
D = 64
G = 4
NB = 4         # 512-wide n-windows
FILL_RATE = [6, 3, 2, 0]   # filler pops per attention chunk-slot, per window


def build_program():
    nc = bacc.Bacc("TRN2", target_bir_lowering=False, debug=False,
                   enable_asserts=False)

    qT = nc.dram_tensor("qT", [E, N], F16, kind="ExternalInput").ap()
    kT = nc.dram_tensor("kT", [E, N], F16, kind="ExternalInput").ap()
    vT = nc.dram_tensor("vT", [E, N], F16, kind="ExternalInput").ap()
    # weight chunk layout: [128, 16*M] — e-chunk ec occupies cols [M*ec, M*ec+M)
    wq = nc.dram_tensor("wq", [128, 16 * 128], F16, kind="ExternalInput").ap()
    wk = nc.dram_tensor("wk", [128, 16 * 128], F16, kind="ExternalInput").ap()
    wv = nc.dram_tensor("wv", [128, 16 * 64], F16, kind="ExternalInput").ap()
    bq2 = nc.dram_tensor("bq2", [128, 1], F32, kind="ExternalInput").ap()
    bk2 = nc.dram_tensor("bk2", [128, 1], F32, kind="ExternalInput").ap()
    bvv = nc.dram_tensor("bvv", [64, 1], F32, kind="ExternalInput").ap()
    wg = nc.dram_tensor("wg", [128, 256], F16, kind="ExternalInput").ap()
    bg01 = nc.dram_tensor("bg01", [128, 1], F32, kind="ExternalInput").ap()
    bg23 = nc.dram_tensor("bg23", [128, 1], F32, kind="ExternalInput").ap()
    wfc = nc.dram_tensor("wfc", [256, E], F16, kind="ExternalInput").ap()
    ident = nc.dram_tensor("ident", [128, 128], F32, kind="ExternalInput").ap()
    out = nc.dram_tensor("out", [N, E], F16, kind="ExternalOutput").ap()

    with tile.TileContext(nc) as tc:
        build_tile_kernel(tc, qT=qT, kT=kT, vT=vT, wq=wq, wk=wk, wv=wv,
                          bq2=bq2, bk2=bk2, bvv=bvv, wg=wg, bg01=bg01,
                          bg23=bg23, wfc=wfc, ident=ident, out=out)
    nc.compile()
    return nc


def build_tile_kernel(tc, *, qT, kT, vT, wq, wk, wv, bq2, bk2, bvv, wg,
                      bg01, bg23, wfc, ident, out):
    nc = tc.nc

    import contextlib
    ctx = contextlib.ExitStack()
    ctx.__enter__()
    cp = ctx.enter_context(tc.tile_pool(name="persist", bufs=1))

    def ptile(shape, dtype, name):
        return cp.tile(shape, dtype, tag=name, name=name)

    # ---- persistent constants / state in SBUF ----
    wq_sb = ptile([128, 16 * 128], F16, "wq_sb")
    wk_sb = ptile([128, 16 * 128], F16, "wk_sb")
    wv_sb = ptile([128, 16 * 64], F16, "wv_sb")
    wg_sb = ptile([128, 256], F16, "wg_sb")
    wfc0_sb = ptile([128, E], F16, "wfc0_sb")
    wfc1_sb = ptile([128, E], F16, "wfc1_sb")
    id_sb = ptile([128, 128], F32, "id_sb")
    bq_sb = ptile([128, 1], F32, "bq_sb")
    bk_sb = ptile([128, 1], F32, "bk_sb")
    bv_sb = ptile([64, 1], F32, "bv_sb")
    bg01_sb = ptile([128, 1], F32, "bg01_sb")
    bg23_sb = ptile([128, 1], F32, "bg23_sb")
    ones_sb = ptile([128, 1], F32, "ones_sb")
    warm_sb = ptile([1, 1], F32, "warm_sb")

    kt_w = [ptile([128, 512], F16, f"kt{j}") for j in range(NB)]
    vo_w = [ptile([128, 4 * 65], F32R, f"vo{j}") for j in range(NB)]

    with ctx:
        in_pool = ctx.enter_context(tc.tile_pool(name="in_pool", bufs=22))
        qt_pool = ctx.enter_context(tc.tile_pool(name="qt_pool", bufs=2))
        qg_pool = ctx.enter_context(tc.tile_pool(name="qg_pool", bufs=2))
        hid_pool = ctx.enter_context(tc.tile_pool(name="hid_pool", bufs=2))
        vt_pool = ctx.enter_context(tc.tile_pool(name="vt_pool", bufs=2))
        pt_pool = ctx.enter_context(tc.tile_pool(name="pt_pool", bufs=3))
        rec_pool = ctx.enter_context(tc.tile_pool(name="rec_pool", bufs=2))
        fco_pool = ctx.enter_context(tc.tile_pool(name="fco_pool", bufs=2))
        misc_ps = ctx.enter_context(
            tc.tile_pool(name="misc_ps", bufs=2, space="PSUM"))
        st_ps = ctx.enter_context(
            tc.tile_pool(name="st_ps", bufs=2, space="PSUM"))
        pv_ps = ctx.enter_context(
            tc.tile_pool(name="pv_ps", bufs=2, space="PSUM"))

        # ---------- shared state set as emission progresses ----------
        in_tiles = {}    # (tensor, pair, batch) -> sbuf tile [128, 2048]
        qg01_w = [None] * NB
        qg23_w = [None] * NB
        hid01_w = [None] * NB
        hid23_w = [None] * NB

        filler = deque()

        def fill(n):
            c = 0
            while filler and c < n:
                filler.popleft()()
                c += 1

        def drain():
            while filler:
                filler.popleft()()

        # ---------- DMA emission helpers ----------
        TSRC = {"q": (qT, "qin"), "k": (kT, "kin"), "v": (vT, "vin")}

        def emit_in_dma(t, w, b, eng):
            """Load window w's slice of e-chunk batch b (chunks 2b, 2b+1):
            a [128, 2, 512] tile.  Per-window tiles die right after their
            window's projection, so later windows' loads never wait long."""
            src_t, tag = TSRC[t]
            ti = in_pool.tile([128, 1024], F16, tag=tag, name=f"{t}in{w}")
            in_tiles[(t, w, b)] = ti
            eng.dma_start(
                ti[:].rearrange("p (c n) -> p c n", c=2),
                src_t[bass.ds(256 * b, 256),
                      bass.ds(512 * w, 512)].rearrange(
                          "(c p) n -> p c n", p=128))

        # ---------- projection emission (per tensor, per window) ----------
        def make_proj_closures(t, j):
            """16 matmuls (8 batch-closures) + 1 bias closure for tensor t,
            window j. Sets qt/kt/vt state."""
            cell = {}

            def mk_mm(b):
                def go():
                    if b == 0:
                        rows = 64 if t == "v" else 128
                        cell["ps"] = misc_ps.tile([rows, 512], F32, tag="mm",
                                                  name=f"{t}_ps")
                    w_sb = {"q": wq_sb, "k": wk_sb, "v": wv_sb}[t]
                    M = 64 if t == "v" else 128
                    ps = cell["ps"]
                    for c in range(2):
                        ec = 2 * b + c
                        mv = in_tiles[(t, j, b)][:, 512 * c: 512 * c + 512]
                        nc.tensor.matmul(ps[:], w_sb[:, bass.ts(ec, M)], mv,
                                         start=(ec == 0), stop=(ec == 15))
                return go

            def bias():
                ps = cell["ps"]
                if t == "q":
                    qt = qt_pool.tile([128, 512], F16, tag="qt", name="qt")
                    nc.scalar.activation(qt[:], ps[:], AF.Identity,
                                         bias=bq_sb[:])
                    cell["qt"] = qt
                elif t == "k":
                    nc.scalar.activation(kt_w[j][:], ps[:], AF.Identity,
                                         bias=bk_sb[:])
                else:
                    vt = vt_pool.tile([64, 512], F32, tag="vt", name="vt")
                    nc.scalar.activation(vt[:], ps[:], AF.Identity,
                                         bias=bv_sb[:])
                    cell["vt"] = vt

            return [mk_mm(b) for b in range(8)] + [bias], cell

        def make_vpath_closures(j, vcell):
            """PE transposes + DVE copies: VT -> vo_w[j] data columns."""
            tr_cell = {}

            def tr():
                tr_ps = misc_ps.tile([128, 256], F32, tag="mm", name="tr_ps")
                for t4 in range(4):
                    nc.tensor.transpose(tr_ps[:, bass.ts(t4, 64)],
                                        vcell["vt"][:, bass.ts(t4, 128)],
                                        id_sb[0:64, 0:64])
                tr_cell["tr"] = tr_ps

            def cp_out():
                vo3 = vo_w[j][:].rearrange("p (t c) -> p t c", c=65)
                for t4 in range(4):
                    nc.vector.tensor_copy(vo3[:, t4, 0:64],
                                          tr_cell["tr"][:, bass.ts(t4, 64)])

            return [tr, cp_out]

        def make_g_closure(j, qcell):
            def go():
                g01 = misc_ps.tile([128, 512], F32, tag="mm", name="g01_ps")
                nc.tensor.matmul(g01[:], wg_sb[0:64, 0:128],
                                 qcell["qt"][0:64, :], start=True, stop=True)
                g23 = misc_ps.tile([128, 512], F32, tag="mm", name="g23_ps")
                nc.tensor.matmul(g23[:], wg_sb[64:128, 128:256],
                                 qcell["qt"][64:128, :], start=True, stop=True)
                qg01 = qg_pool.tile([128, 512], F16, tag="qg01", name="qg01")
                qg23 = qg_pool.tile([128, 512], F16, tag="qg23", name="qg23")
                nc.scalar.activation(qg01[:], g01[:], AF.Identity,
                                     bias=bg01_sb[:])
                nc.scalar.activation(qg23[:], g23[:], AF.Identity,
                                     bias=bg23_sb[:])
                qg01_w[j] = qg01
                qg23_w[j] = qg23
            return go

        def push_in_dmas(w):
            """Queue window-w input loads: q/v on the Sync ring, k on the
            ACT ring."""
            for b in range(8):
                filler.append(lambda b=b: emit_in_dma("q", w, b, nc.sync))
            for b in range(8):
                filler.append(lambda b=b: emit_in_dma("k", w, b, nc.scalar))
            for b in range(8):
                filler.append(lambda b=b: emit_in_dma("v", w, b, nc.sync))

        def push_window_feed(j):
            """Queue proj+G for window j as filler closures."""
            qcl, qcell = make_proj_closures("q", j)
            filler.extend(qcl)
            kcl, _ = make_proj_closures("k", j)
            filler.extend(kcl)
            vcl, vcell = make_proj_closures("v", j)
            filler.extend(vcl)
            filler.extend(make_vpath_closures(j, vcell))
            filler.append(make_g_closure(j, qcell))

        # ---------- FC emission ----------
        def make_fc_closures(j):
            cls = []
            for m in range(4):
                cell = {}
                for eo in range(4):
                    def go(m=m, eo=eo, cell=cell):
                        if eo == 0:
                            cell["stage"] = fco_pool.tile(
                                [128, 2048], F16, tag="stage", name="stage")
                        if j == 3 and (m * 4 + eo) % 2 == 1:
                            fc_ps = st_ps.tile([128, 512], F32, tag="st",
                                               name="fc_ps")
                        else:
                            fc_ps = misc_ps.tile([128, 512], F32, tag="mm",
                                                 name="fc_ps")
                        nc.tensor.matmul(fc_ps[:],
                                         hid01_w[j][:, bass.ts(m, 128)],
                                         wfc0_sb[:, bass.ts(eo, 512)],
                                         start=True, stop=False)
                        nc.tensor.matmul(fc_ps[:],
                                         hid23_w[j][:, bass.ts(m, 128)],
                                         wfc1_sb[:, bass.ts(eo, 512)],
                                         start=False, stop=True)
                        nc.vector.tensor_copy(
                            cell["stage"][:, bass.ts(eo, 512)], fc_ps[:])
                        rows = slice(512 * j + 128 * m, 512 * j + 128 * m + 128)
                        if j == 3:
                            # last window: DMA per-eo on alternating rings so
                            # the final transfers are small and parallel
                            eng = nc.scalar if eo % 2 else nc.sync
                            eng.dma_start(
                                out[rows, bass.ts(eo, 512)],
                                cell["stage"][:, bass.ts(eo, 512)])
                        elif eo == 3:
                            nc.sync.dma_start(out[rows, :], cell["stage"][:])
                    cls.append(go)
            return cls

        # ---------- attention emission ----------
        def emit_window_attn(j):
            K = 4 * j + 4
            for pair, qg_of in ((0, qg01_w), (1, qg23_w)):
                qg = qg_of[j]
                pv_a = pv_ps.tile([65, 512], F32, tag="pv", name="pv_a")
                pv_b = pv_ps.tile([65, 512], F32, tag="pv", name="pv_b")
                pts = {}

                def st_step(k):
                    kt_c = kt_w[k // 4]
                    ks = bass.ts(k % 4, 128)
                    i = k - 4 * j
                    off = max(0, 128 * i)
                    if off == 384:
                        off = 256
                    st = st_ps.tile([128, 1024], F32, tag="st", name="st")
                    nc.tensor.matmul(st[:, off:512], kt_c[0:64, ks],
                                     qg[0:64, off:512], start=True, stop=True)
                    nc.tensor.matmul(st[:, 512 + off:1024], kt_c[64:128, ks],
                                     qg[64:128, off:512],
                                     start=True, stop=True)
                    pt = pt_pool.tile([128, 1024], F32R, tag="pt", name="pt")
                    st3 = st[:].rearrange("p (g c) -> p g c", c=512)
                    pt3 = pt[:].rearrange("p (g c) -> p g c", c=512)
                    nc.scalar.activation(pt3[:, :, off:512],
                                         st3[:, :, off:512],
                                         AF.Exp, scale=8.0)
                    if i >= 0:
                        mw = 128 * i + 128 - off
                        nc.gpsimd.affine_select(
                            out=pt3[:, :, off:off + mw],
                            in_=pt3[:, :, off:off + mw],
                            compare_op=mybir.AluOpType.is_ge,
                            fill=0.0, base=-(128 * i - off),
                            pattern=[[0, 2], [1, mw]],
                            channel_multiplier=-1)
                    pts[k] = (pt, off)

                def pv_step(k):
                    pt, off = pts.pop(k)
                    vo_c = vo_w[k // 4]
                    vsl = vo_c[:, (k % 4) * 65:(k % 4) * 65 + 65]
                    nc.tensor.matmul(pv_a[:, off:512], vsl, pt[:, off:512],
                                     start=(k == 0), stop=(k == K - 1))
                    nc.tensor.matmul(pv_b[:, off:512], vsl,
                                     pt[:, 512 + off:1024],
                                     start=(k == 0), stop=(k == K - 1))

                fill(3 if j == 3 else 2)
                st_step(0)
                if K > 1:
                    st_step(1)
                for k in range(K):
                    if k + 2 < K:
                        st_step(k + 2)
                    # extra filler at the pair start covers the previous
                    # pair's normalize chain before pv psum reuse
                    fill(FILL_RATE[j] + ((3 if k < 1 else (2 if k < 4 else 0)) if j == 3 else (2 if k < 2 else 0)))
                    pv_step(k)

                # normalize: hid[g-half] = pv[0:64] * (1/pv[64])
                if pair == 0:
                    hid = hid_pool.tile([128, 512], F16, tag="hid01",
                                        name="hid01")
                    hid01_w[j] = hid
                else:
                    hid = hid_pool.tile([128, 512], F16, tag="hid23",
                                        name="hid23")
                    hid23_w[j] = hid
                den_a = rec_pool.tile([1, 512], F32, tag="den", name="den_a")
                nc.vector.tensor_copy(den_a[:], pv_a[64:65, :])
                rec_a = rec_pool.tile([1, 512], F32, tag="rec", name="rec_a")
                nc.vector.reciprocal_approx_fast(rec_a[:], den_a[:])
                recr_a = rec_pool.tile([64, 512], F32, tag="recr",
                                       name="recr_a")
                nc.gpsimd.partition_broadcast(recr_a[:], rec_a[:])
                den_b = rec_pool.tile([1, 512], F32, tag="den", name="den_b")
                nc.vector.tensor_copy(den_b[:], pv_b[64:65, :])
                rec_b = rec_pool.tile([1, 512], F32, tag="rec", name="rec_b")
                nc.vector.reciprocal_approx_fast(rec_b[:], den_b[:])
                recr_b = rec_pool.tile([64, 512], F32, tag="recr",
                                       name="recr_b")
                nc.gpsimd.partition_broadcast(recr_b[:], rec_b[:])
                nc.vector.tensor_mul(hid[0:64, :], pv_a[0:64, :], recr_a[:])
                nc.vector.tensor_mul(hid[64:128, :], pv_b[0:64, :],
                                     recr_b[:])

        # ================= prologue =================
        nc.vector.memset(ones_sb[:], 1.0)
        nc.scalar.activation(warm_sb[:], ones_sb[0:1, :], AF.Exp, scale=1.0)
        for j in range(NB):
            for t4 in range(4):
                nc.vector.tensor_copy(
                    vo_w[j][:, t4 * 65 + 64: t4 * 65 + 65], ones_sb[:])

        # pair-0 window-0 halves first (2MB/ring): q on Sync, k/v on ACT ring
        emit_in_dma("q", 0, 0, nc.sync)
        nc.sync.dma_start(wq_sb[:], wq[:])
        for b in range(1, 8):
            emit_in_dma("q", 0, b, nc.sync)
        for b in range(8):
            emit_in_dma("k", 0, b, nc.scalar)
        nc.sync.dma_start(wk_sb[:], wk[:])
        nc.sync.dma_start(bq_sb[:], bq2[:])
        nc.sync.dma_start(bk_sb[:], bk2[:])
        nc.sync.dma_start(wv_sb[:], wv[:])
        nc.sync.dma_start(bv_sb[:], bvv[:])
        nc.sync.dma_start(wg_sb[:], wg[:])
        for b in range(8):
            emit_in_dma("v", 0, b, nc.scalar)
        nc.sync.dma_start(id_sb[:], ident[:])
        nc.sync.dma_start(bg01_sb[:], bg01[:])
        nc.sync.dma_start(bg23_sb[:], bg23[:])
        nc.sync.dma_start(wfc0_sb[:], wfc[0:128, :])
        nc.sync.dma_start(wfc1_sb[:], wfc[128:256, :])

        # window 0 proj + G emitted directly (nothing else to overlap yet)
        for t in ("q", "k", "v"):
            cls, cell = make_proj_closures(t, 0)
            for c in cls:
                c()
            if t == "q":
                q0cell = cell
            if t == "v":
                for c in make_vpath_closures(0, cell):
                    c()
        make_g_closure(0, q0cell)()

        def zip_feed(fc_cls, feed_j):
            """Interleave FC m-groups between whole proj-tensor blocks (a
            projection's psum accumulation must not be interleaved with FC
            psum allocations — both rotate the same "mm" tag)."""
            qcl, qcell = make_proj_closures("q", feed_j)
            kcl, _ = make_proj_closures("k", feed_j)
            vcl, vcell = make_proj_closures("v", feed_j)
            fc_groups = [fc_cls[i:i + 4] for i in range(0, len(fc_cls), 4)]

            def grp(i):
                return fc_groups[i] if i < len(fc_groups) else []

            filler.extend(qcl)
            filler.extend(grp(0))
            filler.extend(kcl)
            filler.extend(grp(1))
            filler.extend(vcl)
            filler.extend(make_vpath_closures(feed_j, vcell))
            filler.extend(grp(2))
            filler.append(make_g_closure(feed_j, qcell))
            for g in fc_groups[3:]:
                filler.extend(g)

        # ================= main pipeline =================
        push_in_dmas(1)
        push_window_feed(1)
        push_in_dmas(2)
        emit_window_attn(0)
        drain()

        push_in_dmas(3)
        fc0 = make_fc_closures(0)
        zip_feed(fc0, 2)
        emit_window_attn(1)
        drain()

        fc1 = make_fc_closures(1)
        zip_feed(fc1[:12], 3)
        emit_window_attn(2)
        drain()

        filler.extend(fc1[12:])
        filler.extend(make_fc_closures(2))
        emit_window_attn(3)
        drain()

        for c in make_fc_closures(3):
            c()


def shard_inputs(inputs):
    """full inputs -> list of 8 per-core in_maps (numpy, device layouts)"""
    f16 = np.float16
    f32 = np.float32
    q = np.asarray(inputs["q"], f32)[0]
    k = np.asarray(inputs["k"], f32)[0]
    v = np.asarray(inputs["v"], f32)[0]
    Wq = np.asarray(inputs["Wq"], f32)
    Wk = np.asarray(inputs["Wk"], f32)
    Wv = np.asarray(inputs["Wv"], f32)
    bq = np.asarray(inputs["bq"], f32)
    bk = np.asarray(inputs["bk"], f32)
    bv = np.asarray(inputs["bv"], f32)
    WG = np.asarray(inputs["WG"], f32)
    bG = np.asarray(inputs["bG"], f32)
    Wfc = np.asarray(inputs["Wfc"], f32)

    qT = np.ascontiguousarray(q.T.astype(f16))
    kT = np.ascontiguousarray(k.T.astype(f16))
    vT = np.ascontiguousarray(v.T.astype(f16))
    ident = np.eye(128, dtype=f32)

    def chunked(w):
        # [E, M] -> [128, 16*M]: e-chunk ec at cols [M*ec, M*ec+M)
        M = w.shape[1]
        return np.ascontiguousarray(
            w.reshape(16, 128, M).transpose(1, 0, 2).reshape(128, 16 * M))

    maps = []
    for h in range(HK):
        sl = slice(h * D, (h + 1) * D)
        wq_h = Wq[:, sl]
        wk_h = Wk[:, sl]
        wv_h = Wv[:, sl]
        m = {
            "qT": qT, "kT": kT, "vT": vT,
            "wq": chunked(np.concatenate([wq_h, wq_h], 1)).astype(f16),
            "wk": chunked(np.concatenate([wk_h, wk_h], 1)).astype(f16),
            "wv": chunked(wv_h).astype(f16),
            "bq2": np.concatenate([bq[sl], bq[sl]]).reshape(128, 1).copy(),
            "bk2": np.concatenate([bk[sl], bk[sl]]).reshape(128, 1).copy(),
            "bvv": bv[sl].reshape(64, 1).copy(),
            "wg": np.concatenate([WG[h], WG[h]], 0).astype(f16),  # [128, 256]
            "bg01": bG[h, 0:128].reshape(128, 1).copy(),
            "bg23": bG[h, 128:256].reshape(128, 1).copy(),
            "wfc": Wfc[h * 256:(h + 1) * 256, :].astype(f16),
            "ident": ident,
        }
        maps.append(m)
    return maps


_compiled = None
last_results = None


def get_compiled():
    global _compiled
    if _compiled is None:
        _compiled = build_program()
    return _compiled


def kernel(**inputs):
    global last_results
    nc = get_compiled()
    in_maps = shard_inputs(inputs)
    last_results = bass_utils.run_bass_kernel_spmd(
        nc, in_maps, core_ids=list(range(8)))
    bfc = np.asarray(inputs["bfc"], np.float32)
    acc = np.zeros((N, E), np.float64)
    for res in last_results.results:
        acc += res["out"].astype(np.float64)
    full = (acc + bfc[None, :].astype(np.float64)).astype(np.float32)
    return full.reshape(1, N, E)


# revision 28
# speedup vs baseline: 1.0804x; 1.0804x over previous
"""CompoundHeadAttention TRN2 kernel (v2 — software-pipelined schedule).

Full-input contract: kernel(**inputs) takes the unsharded tensors from
setup_inputs() and returns the full [1, 2048, 2048] float32 output.

Sharding (8 cores, tensor-parallel over the HK=8 kv heads):
  core h owns kv head h: its Wq/Wk/Wv column slice, its WG[h]/bG[h], and
  Wfc row-slice [h*256:(h+1)*256, :].  Each core computes its head's
  attention + its partial FC output [2048, 2048]; the host sums the 8
  partials and adds bfc (the "all-reduce" of the row-sharded FC).

v2 schedule (vs v1): the PE stream is kept dense — scores (ST) run two
chunks ahead of the PV consumer so the ACT exp latency never stalls the
in-order PE queue; projection/G/FC matmuls are interleaved into the
attention stream as filler at a per-window rate; softmax denominators
use reciprocal_approx_fast (0.7us vs 4us); input DMAs are batched
(2 e-chunks per instr) and issued from both the Sync and ACT queues.
"""

import os
import sys
from collections import deque

import numpy as np

if "/opt/trn_rl_repo" not in sys.path and os.path.isdir("/opt/trn_rl_repo"):
    sys.path.insert(0, "/opt/trn_rl_repo")

import concourse.bass as bass  # noqa: E402
import concourse.mybir as mybir  # noqa: E402
import concourse.tile as tile  # noqa: E402
from concourse import bacc  # noqa: E402
from concourse import bass_utils  # noqa: E402

F32 = mybir.dt.float32
F32R = mybir.dt.float32r
F16 = mybir.dt.float16
AF = mybir.ActivationFunctionType

N = 2048
E = 2048
HK = 8
D = 64
G = 4
NB = 4         # 512-wide n-windows
FILL_RATE = [6, 3, 2, 0]   # filler pops per attention chunk-slot, per window


def build_program():
    nc = bacc.Bacc("TRN2", target_bir_lowering=False, debug=False,
                   enable_asserts=False)

    qT = nc.dram_tensor("qT", [E, N], F16, kind="ExternalInput").ap()
    kT = nc.dram_tensor("kT", [E, N], F16, kind="ExternalInput").ap()
    vT = nc.dram_tensor("vT", [E, N], F16, kind="ExternalInput").ap()
    # weight chunk layout: [128, 16*M] — e-chunk ec occupies cols [M*ec, M*ec+M)
    wq = nc.dram_tensor("wq", [128, 16 * 128], F16, kind="ExternalInput").ap()
    wk = nc.dram_tensor("wk", [128, 16 * 128], F16, kind="ExternalInput").ap()
    wv = nc.dram_tensor("wv", [128, 16 * 64], F16, kind="ExternalInput").ap()
    bq2 = nc.dram_tensor("bq2", [128, 1], F32, kind="ExternalInput").ap()
    bk2 = nc.dram_tensor("bk2", [128, 1], F32, kind="ExternalInput").ap()
    bvv = nc.dram_tensor("bvv", [64, 1], F32, kind="ExternalInput").ap()
    wg = nc.dram_tensor("wg", [128, 256], F16, kind="ExternalInput").ap()
    bg01 = nc.dram_tensor("bg01", [128, 1], F32, kind="ExternalInput").ap()
    bg23 = nc.dram_tensor("bg23", [128, 1], F32, kind="ExternalInput").ap()
    wfc = nc.dram_tensor("wfc", [256, E], F16, kind="ExternalInput").ap()
    ident = nc.dram_tensor("ident", [128, 128], F32, kind="ExternalInput").ap()
    out = nc.dram_tensor("out", [N, E], F16, kind="ExternalOutput").ap()

    with tile.TileContext(nc) as tc:
        build_tile_kernel(tc, qT=qT, kT=kT, vT=vT, wq=wq, wk=wk, wv=wv,
                          bq2=bq2, bk2=bk2, bvv=bvv, wg=wg, bg01=bg01,
                          bg23=bg23, wfc=wfc, ident=ident, out=out)
    nc.compile()
    return nc


def build_tile_kernel(tc, *, qT, kT, vT, wq, wk, wv, bq2, bk2, bvv, wg,
                      bg01, bg23, wfc, ident, out):
    nc = tc.nc

    import contextlib
    ctx = contextlib.ExitStack()
    ctx.__enter__()
    cp = ctx.enter_context(tc.tile_pool(name="persist", bufs=1))

    def ptile(shape, dtype, name):
        return cp.tile(shape, dtype, tag=name, name=name)

    # ---- persistent constants / state in SBUF ----
    wq_sb = ptile([128, 16 * 128], F16, "wq_sb")
    wk_sb = ptile([128, 16 * 128], F16, "wk_sb")
    wv_sb = ptile([128, 16 * 64], F16, "wv_sb")
    wg_sb = ptile([128, 256], F16, "wg_sb")
    wfc0_sb = ptile([128, E], F16, "wfc0_sb")
    wfc1_sb = ptile([128, E], F16, "wfc1_sb")
    id_sb = ptile([128, 128], F32, "id_sb")
    bq_sb = ptile([128, 1], F32, "bq_sb")
    bk_sb = ptile([128, 1], F32, "bk_sb")
    bv_sb = ptile([64, 1], F32, "bv_sb")
    bg01_sb = ptile([128, 1], F32, "bg01_sb")
    bg23_sb = ptile([128, 1], F32, "bg23_sb")
    ones_sb = ptile([128, 1], F32, "ones_sb")
    warm_sb = ptile([1, 1], F32, "warm_sb")

    kt_w = [ptile([128, 512], F16, f"kt{j}") for j in range(NB)]
    vo_w = [ptile([128, 4 * 65], F32R, f"vo{j}") for j in range(NB)]

    with ctx:
        in_pool = ctx.enter_context(tc.tile_pool(name="in_pool", bufs=8))
        qt_pool = ctx.enter_context(tc.tile_pool(name="qt_pool", bufs=2))
        qg_pool = ctx.enter_context(tc.tile_pool(name="qg_pool", bufs=2))
        hid_pool = ctx.enter_context(tc.tile_pool(name="hid_pool", bufs=2))
        vt_pool = ctx.enter_context(tc.tile_pool(name="vt_pool", bufs=2))
        pt_pool = ctx.enter_context(tc.tile_pool(name="pt_pool", bufs=3))
        rec_pool = ctx.enter_context(tc.tile_pool(name="rec_pool", bufs=2))
        fco_pool = ctx.enter_context(tc.tile_pool(name="fco_pool", bufs=2))
        misc_ps = ctx.enter_context(
            tc.tile_pool(name="misc_ps", bufs=2, space="PSUM"))
        st_ps = ctx.enter_context(
            tc.tile_pool(name="st_ps", bufs=2, space="PSUM"))
        pv_ps = ctx.enter_context(
            tc.tile_pool(name="pv_ps", bufs=2, space="PSUM"))

        # ---------- shared state set as emission progresses ----------
        in_tiles = {}    # (tensor, pair, batch) -> sbuf tile [128, 2048]
        qg01_w = [None] * NB
        qg23_w = [None] * NB
        hid01_w = [None] * NB
        hid23_w = [None] * NB

        filler = deque()

        def fill(n):
            c = 0
            while filler and c < n:
                filler.popleft()()
                c += 1

        def drain():
            while filler:
                filler.popleft()()

        # ---------- DMA emission helpers ----------
        TSRC = {"q": (qT, "qin"), "k": (kT, "kin"), "v": (vT, "vin")}

        def emit_in_dma(t, w, b4, eng):
            """Load window w's slice of e-chunk quad b4 (chunks 4b4..4b4+3):
            a [128, 4, 512] tile.  Per-window tiles die right after their
            window's projection, so later windows' loads never wait long."""
            src_t, tag = TSRC[t]
            ti = in_pool.tile([128, 2048], F16, tag=tag, name=f"{t}in{w}")
            in_tiles[(t, w, b4)] = ti
            eng.dma_start(
                ti[:].rearrange("p (c n) -> p c n", c=4),
                src_t[bass.ds(512 * b4, 512),
                      bass.ds(512 * w, 512)].rearrange(
                          "(c p) n -> p c n", p=128))

        # ---------- projection emission (per tensor, per window) ----------
        def make_proj_closures(t, j):
            """16 matmuls (8 batch-closures) + 1 bias closure for tensor t,
            window j. Sets qt/kt/vt state."""
            cell = {}

            def mk_mm(b):
                def go():
                    if b == 0:
                        rows = 64 if t == "v" else 128
                        cell["ps"] = misc_ps.tile([rows, 512], F32, tag="mm",
                                                  name=f"{t}_ps")
                    w_sb = {"q": wq_sb, "k": wk_sb, "v": wv_sb}[t]
                    M = 64 if t == "v" else 128
                    ps = cell["ps"]
                    for c in range(2):
                        ec = 2 * b + c
                        mv = in_tiles[(t, j, ec // 4)][
                            :, 512 * (ec % 4): 512 * (ec % 4) + 512]
                        nc.tensor.matmul(ps[:], w_sb[:, bass.ts(ec, M)], mv,
                                         start=(ec == 0), stop=(ec == 15))
                return go

            def bias():
                ps = cell["ps"]
                if t == "q":
                    qt = qt_pool.tile([128, 512], F16, tag="qt", name="qt")
                    nc.scalar.activation(qt[:], ps[:], AF.Identity,
                                         bias=bq_sb[:])
                    cell["qt"] = qt
                elif t == "k":
                    nc.scalar.activation(kt_w[j][:], ps[:], AF.Identity,
                                         bias=bk_sb[:])
                else:
                    vt = vt_pool.tile([64, 512], F32, tag="vt", name="vt")
                    nc.scalar.activation(vt[:], ps[:], AF.Identity,
                                         bias=bv_sb[:])
                    cell["vt"] = vt

            return [mk_mm(b) for b in range(8)] + [bias], cell

        def make_vpath_closures(j, vcell):
            """PE transposes + DVE copies: VT -> vo_w[j] data columns."""
            tr_cell = {}

            def tr():
                tr_ps = misc_ps.tile([128, 256], F32, tag="mm", name="tr_ps")
                for t4 in range(4):
                    nc.tensor.transpose(tr_ps[:, bass.ts(t4, 64)],
                                        vcell["vt"][:, bass.ts(t4, 128)],
                                        id_sb[0:64, 0:64])
                tr_cell["tr"] = tr_ps

            def cp_out():
                vo3 = vo_w[j][:].rearrange("p (t c) -> p t c", c=65)
                for t4 in range(4):
                    nc.vector.tensor_copy(vo3[:, t4, 0:64],
                                          tr_cell["tr"][:, bass.ts(t4, 64)])

            return [tr, cp_out]

        def make_g_closure(j, qcell):
            def go():
                g01 = misc_ps.tile([128, 512], F32, tag="mm", name="g01_ps")
                nc.tensor.matmul(g01[:], wg_sb[0:64, 0:128],
                                 qcell["qt"][0:64, :], start=True, stop=True)
                g23 = misc_ps.tile([128, 512], F32, tag="mm", name="g23_ps")
                nc.tensor.matmul(g23[:], wg_sb[64:128, 128:256],
                                 qcell["qt"][64:128, :], start=True, stop=True)
                qg01 = qg_pool.tile([128, 512], F16, tag="qg01", name="qg01")
                qg23 = qg_pool.tile([128, 512], F16, tag="qg23", name="qg23")
                nc.scalar.activation(qg01[:], g01[:], AF.Identity,
                                     bias=bg01_sb[:])
                nc.scalar.activation(qg23[:], g23[:], AF.Identity,
                                     bias=bg23_sb[:])
                qg01_w[j] = qg01
                qg23_w[j] = qg23
            return go

        def push_in_dmas(w):
            """Queue window-w input loads: q/v on the Sync ring, k on the
            ACT ring."""
            for b in range(4):
                filler.append(lambda b=b: emit_in_dma("q", w, b, nc.sync))
            for b in range(4):
                filler.append(lambda b=b: emit_in_dma("k", w, b, nc.scalar))
            for b in range(4):
                filler.append(lambda b=b: emit_in_dma("v", w, b, nc.sync))

        def push_window_feed(j):
            """Queue proj+G for window j as filler closures."""
            qcl, qcell = make_proj_closures("q", j)
            filler.extend(qcl)
            kcl, _ = make_proj_closures("k", j)
            filler.extend(kcl)
            vcl, vcell = make_proj_closures("v", j)
            filler.extend(vcl)
            filler.extend(make_vpath_closures(j, vcell))
            filler.append(make_g_closure(j, qcell))

        # ---------- FC emission ----------
        def make_fc_closures(j):
            cls = []
            for m in range(4):
                cell = {}
                for eo in range(4):
                    def go(m=m, eo=eo, cell=cell):
                        if eo == 0:
                            cell["stage"] = fco_pool.tile(
                                [128, 2048], F16, tag="stage", name="stage")
                        if j == 3 and (m * 4 + eo) % 2 == 1:
                            fc_ps = st_ps.tile([128, 512], F32, tag="st",
                                               name="fc_ps")
                        else:
                            fc_ps = misc_ps.tile([128, 512], F32, tag="mm",
                                                 name="fc_ps")
                        nc.tensor.matmul(fc_ps[:],
                                         hid01_w[j][:, bass.ts(m, 128)],
                                         wfc0_sb[:, bass.ts(eo, 512)],
                                         start=True, stop=False)
                        nc.tensor.matmul(fc_ps[:],
                                         hid23_w[j][:, bass.ts(m, 128)],
                                         wfc1_sb[:, bass.ts(eo, 512)],
                                         start=False, stop=True)
                        nc.vector.tensor_copy(
                            cell["stage"][:, bass.ts(eo, 512)], fc_ps[:])
                        rows = slice(512 * j + 128 * m, 512 * j + 128 * m + 128)
                        if j == 3:
                            # last window: DMA per-eo on alternating rings so
                            # the final transfers are small and parallel
                            eng = nc.scalar if eo % 2 else nc.sync
                            eng.dma_start(
                                out[rows, bass.ts(eo, 512)],
                                cell["stage"][:, bass.ts(eo, 512)])
                        elif eo == 3:
                            nc.sync.dma_start(out[rows, :], cell["stage"][:])
                    cls.append(go)
            return cls

        # ---------- attention emission ----------
        def emit_window_attn(j):
            K = 4 * j + 4
            for pair, qg_of in ((0, qg01_w), (1, qg23_w)):
                qg = qg_of[j]
                pv_a = pv_ps.tile([65, 512], F32, tag="pv", name="pv_a")
                pv_b = pv_ps.tile([65, 512], F32, tag="pv", name="pv_b")
                pts = {}

                def st_step(k):
                    kt_c = kt_w[k // 4]
                    ks = bass.ts(k % 4, 128)
                    i = k - 4 * j
                    off = max(0, 128 * i)
                    if off == 384:
                        off = 256
                    st = st_ps.tile([128, 1024], F32, tag="st", name="st")
                    nc.tensor.matmul(st[:, off:512], kt_c[0:64, ks],
                                     qg[0:64, off:512], start=True, stop=True)
                    nc.tensor.matmul(st[:, 512 + off:1024], kt_c[64:128, ks],
                                     qg[64:128, off:512],
                                     start=True, stop=True)
                    pt = pt_pool.tile([128, 1024], F32R, tag="pt", name="pt")
                    st3 = st[:].rearrange("p (g c) -> p g c", c=512)
                    pt3 = pt[:].rearrange("p (g c) -> p g c", c=512)
                    nc.scalar.activation(pt3[:, :, off:512],
                                         st3[:, :, off:512],
                                         AF.Exp, scale=8.0)
                    if i >= 0:
                        mw = 128 * i + 128 - off
                        nc.gpsimd.affine_select(
                            out=pt3[:, :, off:off + mw],
                            in_=pt3[:, :, off:off + mw],
                            compare_op=mybir.AluOpType.is_ge,
                            fill=0.0, base=-(128 * i - off),
                            pattern=[[0, 2], [1, mw]],
                            channel_multiplier=-1)
                    pts[k] = (pt, off)

                def pv_step(k):
                    pt, off = pts.pop(k)
                    vo_c = vo_w[k // 4]
                    vsl = vo_c[:, (k % 4) * 65:(k % 4) * 65 + 65]
                    nc.tensor.matmul(pv_a[:, off:512], vsl, pt[:, off:512],
                                     start=(k == 0), stop=(k == K - 1))
                    nc.tensor.matmul(pv_b[:, off:512], vsl,
                                     pt[:, 512 + off:1024],
                                     start=(k == 0), stop=(k == K - 1))

                fill(3 if j == 3 else 2)
                st_step(0)
                if K > 1:
                    st_step(1)
                for k in range(K):
                    if k + 2 < K:
                        st_step(k + 2)
                    # extra filler at the pair start covers the previous
                    # pair's normalize chain before pv psum reuse
                    fill(FILL_RATE[j] + ((3 if k < 1 else (2 if k < 4 else 0)) if j == 3 else (2 if k < 2 else 0)))
                    pv_step(k)

                # normalize: hid[g-half] = pv[0:64] * (1/pv[64])
                if pair == 0:
                    hid = hid_pool.tile([128, 512], F16, tag="hid01",
                                        name="hid01")
                    hid01_w[j] = hid
                else:
                    hid = hid_pool.tile([128, 512], F16, tag="hid23",
                                        name="hid23")
                    hid23_w[j] = hid
                den_a = rec_pool.tile([1, 512], F32, tag="den", name="den_a")
                nc.vector.tensor_copy(den_a[:], pv_a[64:65, :])
                rec_a = rec_pool.tile([1, 512], F32, tag="rec", name="rec_a")
                nc.vector.reciprocal_approx_fast(rec_a[:], den_a[:])
                recr_a = rec_pool.tile([64, 512], F32, tag="recr",
                                       name="recr_a")
                nc.gpsimd.partition_broadcast(recr_a[:], rec_a[:])
                den_b = rec_pool.tile([1, 512], F32, tag="den", name="den_b")
                nc.vector.tensor_copy(den_b[:], pv_b[64:65, :])
                rec_b = rec_pool.tile([1, 512], F32, tag="rec", name="rec_b")
                nc.vector.reciprocal_approx_fast(rec_b[:], den_b[:])
                recr_b = rec_pool.tile([64, 512], F32, tag="recr",
                                       name="recr_b")
                nc.gpsimd.partition_broadcast(recr_b[:], rec_b[:])
                nc.vector.tensor_mul(hid[0:64, :], pv_a[0:64, :], recr_a[:])
                nc.vector.tensor_mul(hid[64:128, :], pv_b[0:64, :],
                                     recr_b[:])

        # ================= prologue =================
        nc.vector.memset(ones_sb[:], 1.0)
        nc.scalar.activation(warm_sb[:], ones_sb[0:1, :], AF.Exp, scale=1.0)
        for j in range(NB):
            for t4 in range(4):
                nc.vector.tensor_copy(
                    vo_w[j][:, t4 * 65 + 64: t4 * 65 + 65], ones_sb[:])

        # pair-0 window-0 halves first (2MB/ring): q on Sync, k/v on ACT ring
        emit_in_dma("q", 0, 0, nc.sync)
        nc.sync.dma_start(wq_sb[:], wq[:])
        for b in range(1, 4):
            emit_in_dma("q", 0, b, nc.sync)
        for b in range(4):
            emit_in_dma("k", 0, b, nc.scalar)
        nc.sync.dma_start(wk_sb[:], wk[:])
        nc.sync.dma_start(bq_sb[:], bq2[:])
        nc.sync.dma_start(bk_sb[:], bk2[:])
        nc.sync.dma_start(wv_sb[:], wv[:])
        nc.sync.dma_start(bv_sb[:], bvv[:])
        nc.sync.dma_start(wg_sb[:], wg[:])
        for b in range(4):
            emit_in_dma("v", 0, b, nc.scalar)
        nc.sync.dma_start(id_sb[:], ident[:])
        nc.sync.dma_start(bg01_sb[:], bg01[:])
        nc.sync.dma_start(bg23_sb[:], bg23[:])
        nc.sync.dma_start(wfc0_sb[:], wfc[0:128, :])
        nc.sync.dma_start(wfc1_sb[:], wfc[128:256, :])

        # window 0 proj + G emitted directly (nothing else to overlap yet)
        for t in ("q", "k", "v"):
            cls, cell = make_proj_closures(t, 0)
            for c in cls:
                c()
            if t == "q":
                q0cell = cell
            if t == "v":
                for c in make_vpath_closures(0, cell):
                    c()
        make_g_closure(0, q0cell)()

        def zip_feed(fc_cls, feed_j):
            """Interleave FC m-groups between whole proj-tensor blocks (a
            projection's psum accumulation must not be interleaved with FC
            psum allocations — both rotate the same "mm" tag)."""
            qcl, qcell = make_proj_closures("q", feed_j)
            kcl, _ = make_proj_closures("k", feed_j)
            vcl, vcell = make_proj_closures("v", feed_j)
            fc_groups = [fc_cls[i:i + 4] for i in range(0, len(fc_cls), 4)]

            def grp(i):
                return fc_groups[i] if i < len(fc_groups) else []

            filler.extend(qcl)
            filler.extend(grp(0))
            filler.extend(kcl)
            filler.extend(grp(1))
            filler.extend(vcl)
            filler.extend(make_vpath_closures(feed_j, vcell))
            filler.extend(grp(2))
            filler.append(make_g_closure(feed_j, qcell))
            for g in fc_groups[3:]:
                filler.extend(g)

        # ================= main pipeline =================
        push_in_dmas(1)
        push_window_feed(1)
        push_in_dmas(2)
        emit_window_attn(0)
        drain()

        push_in_dmas(3)
        fc0 = make_fc_closures(0)
        zip_feed(fc0, 2)
        emit_window_attn(1)
        drain()

        fc1 = make_fc_closures(1)
        zip_feed(fc1[:12], 3)
        emit_window_attn(2)
        drain()

        filler.extend(fc1[12:])
        filler.extend(make_fc_closures(2))
        emit_window_attn(3)
        drain()

        for c in make_fc_closures(3):
            c()


def shard_inputs(inputs):
    """full inputs -> list of 8 per-core in_maps (numpy, device layouts)"""
    f16 = np.float16
    f32 = np.float32
    q = np.asarray(inputs["q"], f32)[0]
    k = np.asarray(inputs["k"], f32)[0]
    v = np.asarray(inputs["v"], f32)[0]
    Wq = np.asarray(inputs["Wq"], f32)
    Wk = np.asarray(inputs["Wk"], f32)
    Wv = np.asarray(inputs["Wv"], f32)
    bq = np.asarray(inputs["bq"], f32)
    bk = np.asarray(inputs["bk"], f32)
    bv = np.asarray(inputs["bv"], f32)
    WG = np.asarray(inputs["WG"], f32)
    bG = np.asarray(inputs["bG"], f32)
    Wfc = np.asarray(inputs["Wfc"], f32)

    qT = np.ascontiguousarray(q.T.astype(f16))
    kT = np.ascontiguousarray(k.T.astype(f16))
    vT = np.ascontiguousarray(v.T.astype(f16))
    ident = np.eye(128, dtype=f32)

    def chunked(w):
        # [E, M] -> [128, 16*M]: e-chunk ec at cols [M*ec, M*ec+M)
        M = w.shape[1]
        return np.ascontiguousarray(
            w.reshape(16, 128, M).transpose(1, 0, 2).reshape(128, 16 * M))

    maps = []
    for h in range(HK):
        sl = slice(h * D, (h + 1) * D)
        wq_h = Wq[:, sl]
        wk_h = Wk[:, sl]
        wv_h = Wv[:, sl]
        m = {
            "qT": qT, "kT": kT, "vT": vT,
            "wq": chunked(np.concatenate([wq_h, wq_h], 1)).astype(f16),
            "wk": chunked(np.concatenate([wk_h, wk_h], 1)).astype(f16),
            "wv": chunked(wv_h).astype(f16),
            "bq2": np.concatenate([bq[sl], bq[sl]]).reshape(128, 1).copy(),
            "bk2": np.concatenate([bk[sl], bk[sl]]).reshape(128, 1).copy(),
            "bvv": bv[sl].reshape(64, 1).copy(),
            "wg": np.concatenate([WG[h], WG[h]], 0).astype(f16),  # [128, 256]
            "bg01": bG[h, 0:128].reshape(128, 1).copy(),
            "bg23": bG[h, 128:256].reshape(128, 1).copy(),
            "wfc": Wfc[h * 256:(h + 1) * 256, :].astype(f16),
            "ident": ident,
        }
        maps.append(m)
    return maps


_compiled = None
last_results = None


def get_compiled():
    global _compiled
    if _compiled is None:
        _compiled = build_program()
    return _compiled


def kernel(**inputs):
    global last_results
    nc = get_compiled()
    in_maps = shard_inputs(inputs)
    last_results = bass_utils.run_bass_kernel_spmd(
        nc, in_maps, core_ids=list(range(8)))
    bfc = np.asarray(inputs["bfc"], np.float32)
    acc = np.zeros((N, E), np.float64)
    for res in last_results.results:
        acc += res["out"].astype(np.float64)
    full = (acc + bfc[None, :].astype(np.float64)).astype(np.float32)
    return full.reshape(1, N, E)
